# revision 2
# baseline (speedup 1.0000x reference)
"""DigitCaps dynamic-routing kernel for 8 Trainium2 NeuronCores (v2).

Math (reference):
    u_hat[b,c,u,k] = sum_i W[c,u,k,i] * x[b,i,c]          (B=32, I=16, C=8192, U=32, K=16)
    b_ij = 0
    repeat 3x:
        c_ij  = softmax(b_ij, axis=c)
        s     = sum_c c_ij[c,u] * u_hat[b,c,u,k]
        v     = squash(s)    (norm over u, per (b,k))
        b_ij += mean_b <u_hat, v>
    return v

v2 strategy (C sharded 8 ways, C_LOC = 1024/core):
  * W is converted to bf16 host-side and kept RESIDENT in SBUF
    (16.8 MB/core) — it is DMAed from HBM exactly once, instead of the
    5 full f32 streams of the v1 kernel.  HBM traffic drops 172MB->17MB.
  * One fused sweep per routing iteration.  Per 128-channel tile:
      phase A (iters 2,3): 16 VX matmuls  pv_i[c,uk] = sum_b x[b,i,c]v[b,uk]
        (PE, quadrant-packed);  ACT copies PSUM->SBUF bf16;  DVE computes
        prod = pv*W at 2x bf16 rate, folds i with a 4-level in-place
        binary tree (2x) and reduces k with one tensor_reduce;  the
        agreement a[c,u] updates b_state; ACT exponentiates and builds
        e_rep[c,(u,k)] so the W-scale multiply also runs at 2x.
      phase B: DVE scales W into a scratch tile (W stays pristine for the
        next iteration) and the PE runs 16 s-matmuls per tile.
  * s' and Z are exchanged in ONE AllReduce per iteration ([32,544] f32);
    squash is computed redundantly on all 128 partitions.
  * exp() max-subtraction is skipped: b_ij stays within [-0.7,0.7] here.

All matmuls are bf16 (1 PE cycle/row); accumulation is fp32 in PSUM.
"""

import contextlib

import numpy as np
import concourse.bass as bass
import concourse.bacc as bacc
import concourse.tile as tile
import concourse.mybir as mybir
from concourse.bass_utils import run_bass_kernel_spmd

B, I, C, U, K = 32, 16, 8192, 32, 16
UK = U * K
IUK = I * U * K
N_CORES = 8
C_LOC = C // N_CORES
NT = C_LOC // 128
NUM_ITERS = 3

f32 = mybir.dt.float32
bf16 = mybir.dt.bfloat16
MUL = mybir.AluOpType.mult
ADD = mybir.AluOpType.add
Exp = mybir.ActivationFunctionType.Exp

_CACHE = {}


def _declare_io(nc):
    w_in = nc.dram_tensor("w", [C_LOC, IUK], bf16, kind="ExternalInput").ap()
    xn_in = nc.dram_tensor("xn", [128, 4 * C_LOC], bf16, kind="ExternalInput").ap()
    xt_in = nc.dram_tensor("xt", [128, NT * I * B], bf16, kind="ExternalInput").ap()
    v_out = nc.dram_tensor("v_out", [B, UK], f32, kind="ExternalOutput").ap()
    return w_in, xn_in, xt_in, v_out


def _body(nc, w_in, xn_in, xt_in, v_out, fake_cc=False, repeat=1, ig=2):
    IG = ig                      # i's per VX psum tile (IG*UK f32 = 2 banks)
    NG = I // IG
    SZ_F = UK + U                # exchange payload: s' (512) + Z (32)
    tc_pools = [
        ("wpool", dict(bufs=1)),     # resident bf16 W, 128 KiB/partition
        ("xpool", dict(bufs=1)),
        ("spool", dict(bufs=1)),
        ("pvpool", dict(bufs=2)),    # pv copy / prod scratch, 16 KiB each
        ("wscpool", dict(bufs=1)),   # scaled-W scratch, 16 KiB
        ("small", dict(bufs=1)),
        ("pvx", dict(bufs=2, space="PSUM")),
        ("pacc", dict(bufs=1, space="PSUM")),
        ("dram", dict(bufs=1, space="DRAM")),
    ]
    with tile.TileContext(nc) as tc, contextlib.ExitStack() as stack:
        pools = [stack.enter_context(tc.tile_pool(name=n, **kw)) for n, kw in tc_pools]
        wpool, xpool, spool, pvpool, wscpool, small, pvx, pacc, dram = pools

        # ---- persistent tiles ----
        wres = wpool.tile([128, NT * IUK], bf16)
        w5 = wres[:].rearrange("c (n i u k) -> c n i u k", n=NT, i=I, u=U)
        # xn: [(i%4)*32+b, (i//4)*C_LOC+c] = x[b,i,c]/B   (VX stationary)
        xn = xpool.tile([128, 4 * C_LOC], bf16)
        nc.sync.dma_start(xn[:], xn_in[:])
        xn3 = xn[:].rearrange("p (il c) -> p il c", il=4)
        # xt: [cc, (n, i, b)] = x[b, i, n*128+cc]          (s stationary)
        xt = xpool.tile([128, NT * I * B], bf16)
        nc.sync.dma_start(xt[:], xt_in[:])
        xt4 = xt[:].rearrange("c (n i b) -> c n i b", n=NT, i=I)
        ones_b = xpool.tile([128, B], bf16)
        nc.vector.memset(ones_b[:], 1.0)

        b_state = spool.tile([128, NT * U], f32)
        wexp_state = spool.tile([128, NT * U], bf16)
        vrep = spool.tile([128, UK], bf16)

        for rep in range(repeat):
            nc.vector.memset(b_state[:], 0.0)
            for t in range(NUM_ITERS):
                ps_s = pacc.tile([B, UK], f32, tag="ps_s")
                if t == 0:
                    # uniform c_ij: s' = sum_c u_hat (W unscaled), Z == C
                    for n in range(NT):
                        if rep == 0:
                            nc.sync.dma_start(
                                wres[:, bass.ts(n, IUK)], w_in[bass.ts(n, 128), :]
                            )
                        for i in range(I):
                            nc.tensor.matmul(
                                ps_s[:],
                                xt4[:, n, i, :],          # [128c, 32b]
                                w5[:, n, i],              # [128c, U, K]
                                start=(n == 0 and i == 0),
                                stop=(n == NT - 1 and i == I - 1),
                            )
                else:
                    for n in range(NT):
                        # ---- phase A: agreement for this tile ----
                        pv_sb = pvpool.tile([128, IUK], bf16, tag="pv")
                        pv4 = pv_sb[:].rearrange("c (i u k) -> c i u k", i=I, u=U)
                        for g in range(NG):
                            pv2 = pvx.tile([128, IG * UK], f32, tag="pv2")
                            for j in range(IG):
                                i = g * IG + j
                                ih = i % 4
                                nc.tensor.matmul(
                                    pv2[:, bass.ts(j, UK)],
                                    xn3[32 * ih : 32 * (ih + 1), i // 4, bass.ts(n, 128)],
                                    vrep[32 * ih : 32 * (ih + 1), :],
                                    start=True,
                                    stop=True,
                                    tile_position=(32 * ih, 0),
                                )
                            # PSUM f32 -> SBUF bf16 (enables 2x DVE below)
                            nc.scalar.copy(
                                out=pv_sb[:, bass.ts(g, IG * UK)], in_=pv2[:]
                            )
                        # prod = pv * W  (in place, bf16 2x)
                        nc.vector.tensor_tensor(
                            out=pv_sb[:], in0=pv_sb[:], in1=wres[:, bass.ts(n, IUK)],
                            op=MUL,
                        )
                        # fold i: 16 -> 1 (4 in-place tree levels, bf16 2x)
                        half = IUK // 2
                        while half >= UK:
                            nc.vector.tensor_tensor(
                                out=pv_sb[:, :half],
                                in0=pv_sb[:, :half],
                                in1=pv_sb[:, half : 2 * half],
                                op=ADD,
                            )
                            half //= 2
                        # reduce k per u -> a[c,u]
                        a_red = small.tile([128, U], f32, tag="a_red")
                        nc.vector.tensor_reduce(
                            out=a_red[:],
                            in_=pv4[:, 0],
                            axis=mybir.AxisListType.X,
                            op=ADD,
                        )
                        b_slice = b_state[:, bass.ts(n, U)]
                        nc.vector.tensor_tensor(
                            out=b_slice, in0=b_slice, in1=a_red[:], op=ADD
                        )
                        wexp = wexp_state[:, bass.ts(n, U)]
                        nc.scalar.activation(wexp, b_slice, Exp)
                        # e_rep[c,(u,k)] = wexp[c,u] (k-broadcast, via ACT)
                        e_rep = small.tile([128, UK], bf16, tag="e_rep")
                        nc.scalar.copy(
                            out=e_rep[:].rearrange("c (u k) -> c u k", u=U),
                            in_=wexp.broadcast_to([128, U, K]),
                        )
                        # ---- phase B: scale W, s-matmuls ----
                        wsc = wscpool.tile([128, IUK], bf16, tag="wsc")
                        wsc4 = wsc[:].rearrange("c (i u k) -> c i u k", i=I, u=U)
                        e3 = e_rep[:].rearrange("c (u k) -> c u k", u=U)
                        for h in range(4):
                            nc.vector.tensor_tensor(
                                out=wsc4[:, 4 * h : 4 * (h + 1)],
                                in0=w5[:, n, 4 * h : 4 * (h + 1)],
                                in1=e3.unsqueeze(1).broadcast_to([128, 4, U, K]),
                                op=MUL,
                            )
                        for i in range(I):
                            nc.tensor.matmul(
                                ps_s[:],
                                xt4[:, n, i, :],
                                wsc4[:, i],
                                start=(n == 0 and i == 0),
                                stop=(n == NT - 1 and i == I - 1),
                            )

                # ---- exchange: one AllReduce of (s', Z) ----
                sz = small.tile([B, SZ_F], f32, tag="sz")
                if t > 0:
                    # Z partial: ones-matmul over wexp, fold tiles
                    ps_z = pacc.tile([B, NT * U], f32, tag="ps_z")
                    nc.tensor.matmul(
                        ps_z[:], ones_b[:], wexp_state[:], start=True, stop=True
                    )
                    nc.vector.tensor_reduce(
                        out=sz[:, UK:],
                        in_=ps_z[:].rearrange("b (n u) -> b u n", n=NT),
                        axis=mybir.AxisListType.X,
                        op=ADD,
                    )
                else:
                    nc.vector.memset(sz[:, UK:], 0.0)
                nc.scalar.copy(out=sz[:, :UK], in_=ps_s[:])
                cc_in = dram.tile([B, SZ_F], f32, tag="cc_in")
                cc_out = dram.tile([B, SZ_F], f32, tag="cc_out")
                nc.sync.dma_start(cc_in[:], sz[:])
                if fake_cc:
                    nc.sync.dma_start(cc_out[:], cc_in[:])
                else:
                    nc.gpsimd.collective_compute(
                        "AllReduce",
                        ADD,
                        replica_groups=[list(range(N_CORES))],
                        ins=[cc_in.opt()],
                        outs=[cc_out.opt()],
                    )
                # replicate to all 128 partitions (4 groups)
                sz_all = small.tile([128, SZ_F], f32, tag="sz_all")
                for g in range(4):
                    nc.sync.dma_start(sz_all[32 * g : 32 * (g + 1), :], cc_out[:])

                # ---- normalize s, squash into v (on all 128 partitions) ----
                s_n = small.tile([128, UK], f32, tag="s_n")
                if t == 0:
                    nc.scalar.mul(s_n[:], sz_all[:, :UK], 1.0 / C)
                else:
                    rz = small.tile([128, U], f32, tag="rz")
                    nc.vector.reciprocal(rz[:], sz_all[:, UK:])
                    nc.vector.tensor_tensor(
                        out=s_n[:].rearrange("b (u k) -> b u k", u=U),
                        in0=sz_all[:, :UK].rearrange("b (u k) -> b u k", u=U),
                        in1=rz[:].broadcast_to([128, U, K]),
                        op=MUL,
                    )
                sq = small.tile([128, UK], f32, tag="sq")
                nc.vector.tensor_tensor(out=sq[:], in0=s_n[:], in1=s_n[:], op=MUL)
                mag_sq = small.tile([128, K], f32, tag="mag_sq")
                nc.vector.tensor_reduce(
                    out=mag_sq[:],
                    in_=sq[:].rearrange("b (u k) -> b k u", u=U),
                    axis=mybir.AxisListType.X,
                    op=ADD,
                )
                mag = small.tile([128, K], f32, tag="mag")
                nc.scalar.sqrt(mag[:], mag_sq[:])
                den = small.tile([128, K], f32, tag="den")
                nc.vector.tensor_scalar_add(out=den[:], in0=mag_sq[:], scalar1=1.0)
                rden = small.tile([128, K], f32, tag="rden")
                nc.vector.reciprocal(rden[:], den[:])
                # fac = mag_sq / ((1 + mag_sq) * mag) = mag / (1 + mag_sq)
                fac = small.tile([128, K], f32, tag="fac")
                nc.vector.tensor_tensor(out=fac[:], in0=mag[:], in1=rden[:], op=MUL)
                if t < NUM_ITERS - 1:
                    nc.vector.tensor_tensor(
                        out=vrep[:].rearrange("b (u k) -> b k u", u=U),
                        in0=s_n[:].rearrange("b (u k) -> b k u", u=U),
                        in1=fac[:].broadcast_to([128, K, U]),
                        op=MUL,
                    )
                else:
                    v_t = small.tile([B, UK], f32, tag="v_t")
                    nc.vector.tensor_tensor(
                        out=v_t[:].rearrange("b (u k) -> b k u", u=U),
                        in0=s_n[:B, :].rearrange("b (u k) -> b k u", u=U),
                        in1=fac[:B, :].broadcast_to([B, K, U]),
                        op=MUL,
                    )
                    nc.sync.dma_start(v_out[:], v_t[:])


def _build(repeat=1):
    key = ("nc", repeat)
    if key in _CACHE:
        return _CACHE[key]
    nc = bacc.Bacc(
        "TRN2", target_bir_lowering=False, debug=False, num_devices=N_CORES
    )
    w_in, xn_in, xt_in, v_out = _declare_io(nc)
    _body(nc, w_in, xn_in, xt_in, v_out, repeat=repeat)
    nc.compile()
    _CACHE[key] = nc
    return nc


def _prep_inputs(x, W):
    """Shard FULL inputs into the per-core DMA-friendly layouts (bf16)."""
    x = np.asarray(x, dtype=np.float32)
    W = np.asarray(W, dtype=np.float32)
    nbf = mybir.dt.np(bf16)
    in_maps = []
    for r in range(N_CORES):
        # W[c,u,k,i] -> [c, (i,u,k)]
        w_r = np.ascontiguousarray(
            W[r * C_LOC : (r + 1) * C_LOC].transpose(0, 3, 1, 2)
        ).reshape(C_LOC, IUK).astype(nbf)
        xs = x[:, :, r * C_LOC : (r + 1) * C_LOC]  # [B, I, C_LOC] view
        # xn[32*(i%4) + b, (i//4)*C_LOC + c] = xs[b, i, c] / B
        xn_r = (
            np.ascontiguousarray(
                xs.transpose(1, 0, 2).reshape(4, 4, B, C_LOC).transpose(1, 2, 0, 3)
            ).reshape(128, 4 * C_LOC)
            * np.float32(1.0 / B)
        ).astype(nbf)
        # xt[cc, (tile, i, b)] = xs[b, i, tile*128 + cc]
        xt_r = np.ascontiguousarray(
            xs.reshape(B, I, NT, 128).transpose(3, 2, 1, 0)
        ).reshape(128, NT * I * B).astype(nbf)
        in_maps.append({"w": w_r, "xn": xn_r, "xt": xt_r})
    return in_maps


def kernel(x, W):
    nc = _build()
    in_maps = _prep_inputs(x, W)
    # The shared device occasionally wedges (NRT_EXEC_UNIT_UNRECOVERABLE)
    # and recovers on the next attempt — retry once before giving up.
    try:
        res = run_bass_kernel_spmd(nc, in_maps, core_ids=list(range(N_CORES)))
    except Exception:
        import time as _time

        _time.sleep(15)
        res = run_bass_kernel_spmd(nc, in_maps, core_ids=list(range(N_CORES)))
    v = res.results[0]["v_out"]
    return v.reshape(B, U, K, 1).astype(np.float32)


def make_runner(nc, in_maps):
    """Device-resident repeat runner (timing infrastructure for test.py)."""
    import jax
    from concourse import bass2jax
    from concourse.bass2jax import _bass_exec_p, install_neuronx_cc_hook
    from jax.experimental.shard_map import shard_map
    from jax.sharding import Mesh, PartitionSpec, NamedSharding

    install_neuronx_cc_hook()
    n_cores = len(in_maps)
    partition_name = nc.partition_id_tensor.name if nc.partition_id_tensor else None
    in_names, out_names, out_avals, zero_outs = [], [], [], []
    for alloc in nc.m.functions[0].allocations:
        if not isinstance(alloc, mybir.MemoryLocationSet):
            continue
        name = alloc.memorylocations[0].name
        if alloc.kind == "ExternalInput":
            if name != partition_name:
                in_names.append(name)
        elif alloc.kind == "ExternalOutput":
            out_names.append(name)
            shape = tuple(alloc.tensor_shape)
            dtype = mybir.dt.np(alloc.dtype)
            out_avals.append(jax.core.ShapedArray(shape, dtype))
            zero_outs.append(np.zeros(shape, dtype))
    n_params = len(in_names)
    n_outs = len(out_avals)
    all_in_names = list(in_names) + out_names
    if partition_name is not None:
        all_in_names.append(partition_name)

    def _bd(*args):
        operands = list(args)
        if partition_name is not None:
            operands.append(bass2jax.partition_id_tensor())
        outs = _bass_exec_p.bind(
            *operands,
            out_avals=tuple(out_avals),
            in_names=tuple(all_in_names),
            out_names=tuple(out_names),
            lowering_input_output_aliases=(),
            sim_require_finite=True,
            sim_require_nnan=True,
            nc=nc,
        )
        return tuple(outs)

    devices = jax.devices()[:n_cores]
    mesh = Mesh(np.asarray(devices), ("core",))
    in_specs = (PartitionSpec("core"),) * (n_params + n_outs)
    out_specs = (PartitionSpec("core"),) * len(out_names)
    donate = tuple(range(n_params, n_params + n_outs))
    sharded = jax.jit(
        shard_map(
            _bd, mesh=mesh, in_specs=in_specs, out_specs=out_specs, check_rep=False
        ),
        donate_argnums=donate,
        keep_unused=True,
    )
    sh = NamedSharding(mesh, PartitionSpec("core"))
    concat_in = [
        jax.device_put(
            np.concatenate([np.asarray(in_maps[c][nm]) for c in range(n_cores)], 0),
            sh,
        )
        for nm in in_names
    ]
    for a in concat_in:
        a.block_until_ready()

    def run(n_iter=1):
        outs = None
        for _ in range(n_iter):
            zeros = [
                np.zeros((n_cores * z.shape[0], *z.shape[1:]), z.dtype)
                for z in zero_outs
            ]
            outs = sharded(*concat_in, *zeros)
        for o in outs:
            o.block_until_ready()
        return outs

    return run
